# revision 22
# baseline (speedup 1.0000x reference)
"""Baichuan attention (B=2, S=1024, H=5120, NH=40, fp32) on 8 trn2 NeuronCores.

Strategy: tensor-parallel over heads (5 heads/core). Each core computes
qkv^T for its heads (fp16 matmuls, fp32 PSUM accumulate), causal+alibi
attention without max-subtraction (exp args are small; probs scaled by
1/64 to stay in fp16 range), and a partial o_proj over its 640
contraction dims. The 8 partial outputs are summed on the host.

The alibi mask is never shipped: slopes are derived from the mask input
on the host (mask[h, q, k] = causal + slope_h * k) and turned into
per-partition bias vectors for the exp activation; causality is handled
by only computing k-tiles at or below the diagonal plus a gpsimd
triangular zero-fill on the diagonal probability tile.

Windowed alibi attention: exp(slope*(k-q)) decays so fast that heads
with large slopes only attend a short distance back. Each core has five
head SLOTS with fixed k-tile windows (8,8,3,2,2) -- the same program on
every core (SPMD) -- and the host assigns heads to slots so every
head's required window fits. Skipped k-tiles change the at/zz PSUM
accumulation start flags only; ranges are nested so flags stay sound.

Softmax denominator: Z per q via ones-matmul (PSUM), then 1/Z as
exp(-ln Z) on the scalar engine (the DVE reciprocal is ~6 cycles/elem
on one lane), broadcast across partitions with a rank-1 matmul, and a
single DVE multiply into fp16 attnt.

o_proj keeps W_o^T 128x128 blocks stationary and streams attnt tokens
through them (out^T layout, host transposes): each LDWEIGHTS is
amortized over 1024 moving columns and PSUM needs only 2 rotating
banks (blk ping-pong), so the drain runs near peak.

All device-side layouts put the matmul contraction dim on partitions:
  xt    [B, 128, KT, S]        x^T tiles (partition = hidden dim within k-tile)
  wqkv  [3*HPC, 128, KT, 128]  W_pack^T strips per output m-tile
  wo2   [OC2, HPC, 128, 128]   W_o^T blocks (partition = contraction dim)
  out   [OC2, 128, B*S]        out^T partial, fp16 (output dims on partitions)
"""

import math
import os
from contextlib import ExitStack
from dataclasses import dataclass

import numpy as np

import concourse.bass as bass
import concourse.mybir as mybir
from concourse import bacc
import concourse.tile as tile
from concourse import masks
from concourse.bass_utils import run_bass_kernel_spmd

F16 = mybir.dt.float16
F32 = mybir.dt.float32
P = 128
SCALE = 1.0 / math.sqrt(128.0)
LN_PSCALE = math.log(64.0)  # probs scaled by 1/64 so fp16 never overflows
WIN_TOL = 5e-4  # max truncated softmax mass per head
DEFAULT_SLOTS = (8, 8, 3, 2, 2)


@dataclass(frozen=True)
class Cfg:
    B: int = 2
    S: int = 1024
    KT: int = 40  # contraction tiles; H = KT * 128
    HPC: int = 5  # heads per core
    n_cores: int = 8

    @property
    def H(self):
        return self.KT * P

    @property
    def QT(self):
        return self.S // P

    @property
    def MQKV(self):
        return 3 * self.HPC

    @property
    def NBLK(self):
        return self.S // 512

    @property
    def OC2(self):
        return self.H // P


FULL = Cfg()


def build_nc(cfg: Cfg, slots: tuple) -> bass.Bass:
    nc = bacc.Bacc("TRN2", debug=False)
    B, S, KT, HPC, QT, MQKV = cfg.B, cfg.S, cfg.KT, cfg.HPC, cfg.QT, cfg.MQKV
    OC2, NBLK = cfg.OC2, cfg.NBLK
    KPB = 512 // P  # k-tiles per 512-wide q block

    xt_d = nc.dram_tensor("xt", [B, P, KT, S], F16, kind="ExternalInput")
    ws_d = nc.dram_tensor("wqkv", [MQKV, P, KT, P], F16, kind="ExternalInput")
    wo_d = nc.dram_tensor("wo2", [OC2, P, HPC, P], F16, kind="ExternalInput")
    bias_d = nc.dram_tensor("bias", [P, HPC * QT], F32, kind="ExternalInput")
    qramp_d = nc.dram_tensor("qramp", [1, S], F16, kind="ExternalInput")
    slc_d = nc.dram_tensor("slc", [1, HPC * P], F16, kind="ExternalInput")
    out_d = nc.dram_tensor("out", [OC2, P, B * S], F16, kind="ExternalOutput")

    with ExitStack() as ctx:
        tc = ctx.enter_context(tile.TileContext(nc))
        consts = ctx.enter_context(tc.tile_pool(name="consts", bufs=1))
        xt_pool = ctx.enter_context(tc.tile_pool(name="xt", bufs=1))
        wqkv_pool = ctx.enter_context(tc.tile_pool(name="wqkv", bufs=2))
        qkvt_pool = ctx.enter_context(tc.tile_pool(name="qkvt", bufs=2))
        v_pool = ctx.enter_context(tc.tile_pool(name="v", bufs=6))
        p_pool = ctx.enter_context(tc.tile_pool(name="p", bufs=4))
        attnt_pool = ctx.enter_context(tc.tile_pool(name="attnt", bufs=2))
        norm_pool = ctx.enter_context(tc.tile_pool(name="norm", bufs=2))
        vt_pool = ctx.enter_context(tc.tile_pool(name="vt", bufs=2))
        wo_pool = ctx.enter_context(tc.tile_pool(name="wo", bufs=3))
        out_pool = ctx.enter_context(tc.tile_pool(name="out", bufs=4))
        # PSUM budget (8 banks): ps 2 + sc 2 + at 1 + zz 1 + po 2
        ps_pool = ctx.enter_context(tc.tile_pool(name="ps", bufs=2, space="PSUM"))
        sc_pool = ctx.enter_context(tc.tile_pool(name="sc", bufs=2, space="PSUM"))
        at_pool = ctx.enter_context(tc.tile_pool(name="at", bufs=1, space="PSUM"))
        zz_pool = ctx.enter_context(tc.tile_pool(name="zz", bufs=1, space="PSUM"))
        po_pool = ctx.enter_context(tc.tile_pool(name="po", bufs=2, space="PSUM"))

        # constants
        ident = consts.tile([P, P], F16)
        masks.make_identity(nc, ident[:])
        ones = consts.tile([P, 1], F16)
        nc.gpsimd.memset(ones[:], 1.0)
        ones_row = consts.tile([1, P], F16)
        nc.gpsimd.memset(ones_row[:], 1.0)
        bias_sb = consts.tile([P, HPC * QT], F32)
        nc.sync.dma_start(bias_sb[:], bias_d[:])
        qr_sb = consts.tile([1, S], F16)
        nc.sync.dma_start(qr_sb[:], qramp_d[:])
        slc_sb = consts.tile([1, HPC * P], F16)
        nc.sync.dma_start(slc_sb[:], slc_d[:])

        # PE warm-up: self-contained matmuls keep the PE ramping to full
        # p-state while the first input DMAs stream
        warm = ps_pool.tile([P, 512], F32, tag="ps", name="warm")
        for _ in range(40):
            nc.tensor.matmul(warm[:, :P], ident[:], ident[:], start=True, stop=True)

        if KT >= 8:
            sizes = [1, 2, 4, 5]
            rem = KT - sum(sizes)
            nrem = 4
            q, r = divmod(rem, nrem)
            sizes += [q + (1 if i < r else 0) for i in range(nrem)]
        else:
            sizes = [1] * KT
        k2chunk = []
        for ci, s in enumerate(sizes):
            for j in range(s):
                k2chunk.append((ci, j))
        state = {}

        def load_xt(b):
            # chunk tiles with progressive sizes: QKV starts as soon as the
            # first small chunk lands instead of after the full 10MB
            xt_ch = []
            c0 = 0
            for ci, s in enumerate(sizes):
                xc = xt_pool.tile([P, s, S], F16, tag=f"xt{ci}", name=f"xt{ci}")
                nc.sync.dma_start(xc[:], xt_d[b, :, c0 : c0 + s, :])
                xt_ch.append(xc)
                c0 += s
            state[b, "xt"] = xt_ch

        def prefetch_ws(b, m):
            if m >= MQKV:
                b, m = b + 1, 0
            if b >= B:
                return
            ws = wqkv_pool.tile([P, KT, P], F16, tag="ws", name=f"ws{b}_{m}")
            nc.sync.dma_start(ws[:], ws_d[m])
            state[b, "ws", m] = ws

        def qkv_mtile(b, m):
            # one 128-row strip of qkv^T = W^T.T @ x^T (contraction over H);
            # hf-sequential so only one PSUM bank is held at a time
            if (b, "qkvt") not in state:
                state[b, "qkvt"] = qkvt_pool.tile(
                    [P, 2 * HPC, S], F16, tag="qkvt", name=f"qkvt{b}"
                )
            qkvt_sb = state[b, "qkvt"]
            xt_ch = state[b, "xt"]
            ws = state.pop((b, "ws", m))
            prefetch_ws(b, m + 1)
            if m >= 2 * HPC:
                vt = vt_pool.tile([P, S], F16, tag="vt", name=f"vt{b}_{m}")

            def dst_of(hf):
                if m < 2 * HPC:
                    return qkvt_sb[:, m, hf * 512 : (hf + 1) * 512]
                return vt[:, hf * 512 : (hf + 1) * 512]

            if m == 0:
                # first m-tile is paced by the xt DMA: consume each k-tile
                # for both halves as it arrives
                pss = [
                    ps_pool.tile([P, 512], F32, tag="ps", name=f"ps{b}_{m}_{hf}")
                    for hf in range(S // 512)
                ]
                for k in range(KT):
                    ci, cj = k2chunk[k]
                    for hf in range(S // 512):
                        nc.tensor.matmul(
                            pss[hf][:],
                            ws[:, k, :],
                            xt_ch[ci][:, cj, hf * 512 : (hf + 1) * 512],
                            start=(k == 0),
                            stop=(k == KT - 1),
                        )
                    if k % 4 == 3:
                        yield
                for hf in range(S // 512):
                    nc.scalar.activation(
                        dst_of(hf), pss[hf][:], mybir.ActivationFunctionType.Copy
                    )
                yield
            else:
                for hf in range(S // 512):
                    ps = ps_pool.tile(
                        [P, 512], F32, tag="ps", name=f"ps{b}_{m}_{hf}"
                    )
                    for k in range(KT):
                        ci, cj = k2chunk[k]
                        nc.tensor.matmul(
                            ps[:],
                            ws[:, k, :],
                            xt_ch[ci][:, cj, hf * 512 : (hf + 1) * 512],
                            start=(k == 0),
                            stop=(k == KT - 1),
                        )
                        if k % 4 == 3:
                            yield
                    nc.scalar.activation(
                        dst_of(hf), ps[:], mybir.ActivationFunctionType.Copy
                    )
                    yield
            if m >= 2 * HPC:
                # v^T strip: PE-transpose to per-head natural V
                hh = m - 2 * HPC
                v_sb = v_pool.tile([P, QT, P], F16, tag="v", name=f"v{b}_{hh}")
                state[b, "v", hh] = v_sb
                for i in range(QT):
                    tp = sc_pool.tile([P, P], F16, tag="sc")
                    nc.tensor.transpose(tp[:], vt[:, i * P : (i + 1) * P], ident[:])
                    nc.vector.tensor_copy(v_sb[:, i, :], tp[:])
                    if i % 2 == 1:
                        yield

        def finish_norm(b, hh, blk, iz, at_sb):
            # deferred normalize finisher: broadcast 1/Z and multiply; issued
            # a few tiles into the NEXT blk so the PE queue never waits on
            # the reciprocal
            attnt_sb = state[b, "attnt"]
            bc = sc_pool.tile([P, 512], F32, tag="sc")
            nc.tensor.matmul(bc[:], ones_row[:], iz[:], start=True, stop=True)
            izb = norm_pool.tile([P, 512], F16, tag="izb")
            nc.scalar.activation(izb[:], bc[:], mybir.ActivationFunctionType.Copy)
            nc.vector.tensor_tensor(
                attnt_sb[:, hh, blk * 512 : (blk + 1) * 512],
                at_sb[:],
                izb[:],
                mybir.AluOpType.mult,
            )

        def drain_norm():
            pending = state.get("norm_pending")
            if pending:
                finish_norm(*pending)
                state["norm_pending"] = None

        def attn_head(b, hh):
            # scores^T = K^T.T @ Q^T with k-positions on partitions; windowed
            # ragged tiles; p = exp(s/sqrt(d) + alibi_k - slope*q - ln64)
            W = slots[hh]
            if (b, "attnt") not in state:
                state[b, "attnt"] = attnt_pool.tile(
                    [P, HPC, S], F16, tag="attnt", name=f"attnt{b}"
                )
            attnt_sb = state[b, "attnt"]
            qkvt_sb = state[b, "qkvt"]
            v_sb = state[b, "v", hh]
            for blk in range(NBLK):
                i_first = max(0, KPB * blk - W + 1)
                i_last = KPB * (blk + 1) - 1
                at = at_pool.tile([P, 512], F32, tag="at")
                zz = zz_pool.tile([1, 512], F32, tag="zz")
                prev = None

                def flush(tile_info):
                    pt, pc0, pw, pi = tile_info
                    o0 = pc0 - blk * 512
                    nc.tensor.matmul(
                        zz[:, o0:512],
                        ones[:],
                        pt[:, :pw],
                        start=(pi == i_first),
                        stop=(pi == i_last),
                    )
                    nc.tensor.matmul(
                        at[:, o0:512],
                        v_sb[:, pi, :],
                        pt[:, :pw],
                        start=(pi == i_first),
                        stop=(pi == i_last),
                    )

                for idx, i in enumerate(range(i_first, i_last + 1)):
                    if idx == 2:
                        drain_norm()
                    k0 = i * P
                    c0 = max(blk * 512, k0)
                    c1 = (blk + 1) * 512
                    w = c1 - c0
                    sc = sc_pool.tile([P, 512], F32, tag="sc")
                    nc.tensor.matmul(
                        sc[:, :w],
                        qkvt_sb[:, HPC + hh, k0 : k0 + P],
                        qkvt_sb[:, hh, c0:c1],
                        start=True,
                        stop=False,
                    )
                    # per-q stabilizer: scores += -slope*q/SCALE (rank-1; any
                    # per-q shift cancels in the softmax normalization)
                    nc.tensor.matmul(
                        sc[:, :w],
                        slc_sb[:, hh * P : (hh + 1) * P],
                        qr_sb[:, c0:c1],
                        start=False,
                        stop=True,
                    )
                    pt = p_pool.tile([P, 512], F16, tag="p")
                    nc.scalar.activation(
                        pt[:, :w],
                        sc[:, :w],
                        mybir.ActivationFunctionType.Exp,
                        bias=bias_sb[:, hh * QT + i : hh * QT + i + 1],
                        scale=SCALE,
                    )
                    if c0 == k0:
                        # diagonal tile: zero probs above the diagonal (on the
                        # idle gpsimd queue; DVE is busy with reciprocals)
                        nc.gpsimd.affine_select(
                            out=pt[:, :P],
                            in_=pt[:, :P],
                            compare_op=mybir.AluOpType.is_ge,
                            fill=0.0,
                            base=0,
                            pattern=[[1, P]],
                            channel_multiplier=-1,
                        )
                    if prev is not None:
                        flush(prev)
                    prev = (pt, c0, w, i)
                    yield
                flush(prev)
                drain_norm()
                # copy at/zz out of PSUM immediately (frees both banks for
                # the next blk); reciprocal runs on the SBUF copies
                at_sb = norm_pool.tile([P, 512], F16, tag="atsb")
                nc.scalar.activation(
                    at_sb[:], at[:], mybir.ActivationFunctionType.Copy
                )
                zz_sb = norm_pool.tile([1, 512], F32, tag="zzsb")
                nc.scalar.activation(
                    zz_sb[:], zz[:], mybir.ActivationFunctionType.Copy
                )
                iz = norm_pool.tile([1, 512], F16, tag="iz")
                with nc.allow_low_precision("1/Z in f16 is well within tol"):
                    nc.vector.reciprocal(iz[:], zz_sb[:])
                state["norm_pending"] = (b, hh, blk, iz, at_sb)
                yield

        oproj_order = [(0, oc) for oc in range(OC2)] + [
            (1, oc) for oc in range(OC2)
        ]

        def prefetch_wo(idx):
            if idx >= len(oproj_order):
                return
            b, oc = oproj_order[idx]
            wt = wo_pool.tile([P, HPC, P], F16, tag="wo", name=f"wo{b}_{oc}")
            nc.sync.dma_start(wt[:], wo_d[oc])
            state["wo", b, oc] = wt

        def oproj_chunk(idx):
            # out^T[oc, tok] partial: W_o^T blocks stationary, attnt moving
            b, oc = oproj_order[idx]
            attnt_sb = state[b, "attnt"]
            wt = state.pop(("wo", b, oc))
            prefetch_wo(idx + 2)
            for blk in range(NBLK):
                po = po_pool.tile([P, 512], F32, tag="po")
                for k in range(HPC):
                    nc.tensor.matmul(
                        po[:],
                        wt[:, k, :],
                        attnt_sb[:, k, blk * 512 : (blk + 1) * 512],
                        start=(k == 0),
                        stop=(k == HPC - 1),
                    )
                    if k == 2:
                        yield
                ot = out_pool.tile([P, 512], F16, tag="ot")
                if blk % 2 == 0:
                    nc.scalar.activation(
                        ot[:], po[:], mybir.ActivationFunctionType.Copy
                    )
                else:
                    nc.vector.tensor_copy(ot[:], po[:])
                nc.sync.dma_start(
                    out_d[oc, :, b * S + blk * 512 : b * S + (blk + 1) * 512],
                    ot[:],
                )
                yield

        class Stepper:
            def __init__(self, gens):
                self.gens = list(gens)
                self.i = 0

            def step(self):
                while self.i < len(self.gens):
                    try:
                        next(self.gens[self.i])
                        return True
                    except StopIteration:
                        self.i += 1
                return False

            def drain(self):
                while self.step():
                    pass

        def weave(primaries, filler, ratio):
            for g in primaries:
                for _ in g:
                    for _ in range(ratio):
                        if not filler.step():
                            break

        # ---- software pipeline
        prefetch_ws(0, 0)
        load_xt(0)
        Stepper([qkv_mtile(0, m) for m in range(MQKV)]).drain()
        load_xt(1)
        qkv1 = Stepper([qkv_mtile(1, m) for m in range(MQKV)])
        weave([attn_head(0, hh) for hh in range(HPC)], qkv1, ratio=3)
        drain_norm()
        prefetch_wo(0)
        prefetch_wo(1)
        weave([oproj_chunk(i) for i in range(OC2 - 20)], qkv1, ratio=1)
        qkv1.drain()
        tail0 = Stepper([oproj_chunk(i) for i in range(OC2 - 20, OC2)])
        weave([attn_head(1, hh) for hh in range(HPC)], tail0, ratio=2)
        drain_norm()
        tail0.drain()
        Stepper([oproj_chunk(i) for i in range(OC2, 2 * OC2)]).drain()

    nc.compile()
    return nc


def head_windows(slopes):
    """Per-head minimum k-tile window (2/3/4) or 8 (full) from alibi slopes."""
    req = []
    for s in slopes:
        w = 8
        if s > 0:
            for cand in (2, 3, 4):
                g = 128 * cand - 127
                mass = math.exp(-g * s) / max(1e-30, 1.0 - math.exp(-s))
                if mass <= WIN_TOL:
                    w = cand
                    break
        req.append(w)
    return req


def assign_heads(slopes, cfg: Cfg, slot_shape=DEFAULT_SLOTS):
    """Assign heads to per-core slots so each head's window fits its slot.
    Returns (heads_per_core, slots) -- falls back to full windows if the
    slopes don't fit the default slot shape."""
    NH = len(slopes)
    req = head_windows(slopes)
    for slots in (slot_shape, (8,) * cfg.HPC):
        all_slots = [
            (c, k) for c in range(cfg.n_cores) for k in range(cfg.HPC)
        ]
        # place most demanding heads first, into the tightest fitting slot
        order = sorted(range(NH), key=lambda h: -req[h])
        free = sorted(all_slots, key=lambda s: slots[s[1]])
        placement = {}
        ok = True
        for h in order:
            pick = None
            for idx, (c, k) in enumerate(free):
                if slots[k] >= req[h]:
                    pick = idx
                    break
            if pick is None:
                ok = False
                break
            placement[free.pop(pick)] = h
        if ok:
            heads_per_core = [
                [placement[(c, k)] for k in range(cfg.HPC)]
                for c in range(cfg.n_cores)
            ]
            return heads_per_core, tuple(slots)
    raise AssertionError("unreachable: full windows always fit")


def prep_inputs(hidden_states, W_pack, W_o, attention_mask, cfg: Cfg = FULL):
    """Shard + lay out the full inputs for the 8 cores."""
    B, S, KT, HPC = cfg.B, cfg.S, cfg.KT, cfg.HPC
    H = cfg.H
    hs = np.asarray(hidden_states)
    wp = np.asarray(W_pack)
    wo = np.asarray(W_o)
    am = np.asarray(attention_mask)

    # x^T layout [B, 128, KT, S]: xt[b, p, k, t] = hs[b, t, k*128 + p]
    xt = np.ascontiguousarray(
        hs.reshape(B, S, KT, P).transpose(0, 3, 2, 1).astype(np.float16)
    )

    # alibi slopes from the mask: mask[h, q, k] = causal + slope_h * k
    slopes = am[:, -1, 1].astype(np.float64)  # mask[h, S-1, 1] = slope_h
    if os.environ.get("BAI_NOWIN"):
        slots = (8,) * cfg.HPC
        heads_per_core = [
            list(range(c * cfg.HPC, (c + 1) * cfg.HPC))
            for c in range(cfg.n_cores)
        ]
    else:
        heads_per_core, slots = assign_heads(slopes, cfg)

    kvec = np.arange(P, dtype=np.float64)
    in_maps = []
    for c in range(cfg.n_cores):
        heads = heads_per_core[c]
        # W_pack^T strips: m-tiles [q0..q4, k0..k4, v0..v4] for this core's heads
        rows = []
        for sec in range(3):  # q, k, v blocks of W_pack
            for h in heads:
                r0 = sec * H + h * P
                rows.append(wp[r0 : r0 + P, :])  # [128, H]
        # strip[m, p, k, j] = W_pack[row_j, k*128 + p]
        ws = np.stack(
            [r.T.reshape(KT, P, P).transpose(1, 0, 2) for r in rows]
        ).astype(np.float16)

        # W_o^T blocks: wo2[oc, p, k, o] = W_o[oc*128 + o, heads[k]*128 + p]
        # (dram layout matches the SBUF tile [P, HPC, P] exactly)
        wo2 = np.empty((cfg.OC2, P, HPC, P), dtype=np.float16)
        for k, h in enumerate(heads):
            cols = wo[:, h * P : (h + 1) * P]  # [H, 128]
            wo2[:, :, k, :] = cols.reshape(cfg.OC2, P, P).transpose(0, 2, 1)

        # exp bias table [128, HPC*QT]: col hh*QT + i -> slope*(i*128+k) - lnPS
        bias = np.empty((P, HPC * cfg.QT), dtype=np.float32)
        slc = np.empty((1, HPC * P), dtype=np.float16)
        for hh, h in enumerate(heads):
            for i in range(cfg.QT):
                bias[:, hh * cfg.QT + i] = (
                    slopes[h] * (i * P + kvec) - LN_PSCALE
                ).astype(np.float32)
            slc[0, hh * P : (hh + 1) * P] = np.float16(slopes[h])
        qramp = (
            -np.arange(S, dtype=np.float64) * math.sqrt(128.0)
        ).astype(np.float16)[None, :]

        in_maps.append(
            {
                "xt": xt,
                "wqkv": np.ascontiguousarray(ws),
                "wo2": np.ascontiguousarray(wo2),
                "bias": bias,
                "qramp": qramp,
                "slc": slc,
            }
        )
    return in_maps, slots


_CACHE = {}


def _get_nc(cfg: Cfg, slots: tuple) -> bass.Bass:
    key = (cfg, slots)
    if key not in _CACHE:
        _CACHE[key] = build_nc(cfg, slots)
    return _CACHE[key]


def run(hidden_states, W_pack, W_o, attention_mask, cfg: Cfg = FULL, **kw):
    in_maps, slots = prep_inputs(hidden_states, W_pack, W_o, attention_mask, cfg)
    nc = _get_nc(cfg, slots)
    res = run_bass_kernel_spmd(nc, in_maps, core_ids=list(range(cfg.n_cores)), **kw)
    # sum the per-core partials (fp16 -> fp32), unshard out^T to [B, S, H]
    acc = np.zeros((cfg.OC2, P, cfg.B * cfg.S), dtype=np.float32)
    for r in res.results:
        acc += r["out"].astype(np.float32)
    out = acc.reshape(cfg.H, cfg.B, cfg.S).transpose(1, 2, 0)
    return np.ascontiguousarray(out), res


def kernel(hidden_states, W_pack, W_o, attention_mask):
    out, _ = run(hidden_states, W_pack, W_o, attention_mask)
    return out.astype(np.float32)


# revision 26
# speedup vs baseline: 1.0413x; 1.0413x over previous
"""Baichuan attention (B=2, S=1024, H=5120, NH=40, fp32) on 8 trn2 NeuronCores.

Strategy: tensor-parallel over heads (5 heads/core). Each core computes
qkv^T for its heads (fp16 matmuls, fp32 PSUM accumulate), causal+alibi
attention without max-subtraction (exp args are small; probs scaled by
1/64 to stay in fp16 range), and a partial o_proj over its 640
contraction dims. The 8 partial outputs are summed on the host.

The alibi mask is never shipped: slopes are derived from the mask input
on the host (mask[h, q, k] = causal + slope_h * k) and turned into
per-partition bias vectors for the exp activation; causality is handled
by only computing k-tiles at or below the diagonal plus a gpsimd
triangular zero-fill on the diagonal probability tile.

Windowed alibi attention: exp(slope*(k-q)) decays so fast that heads
with large slopes only attend a short distance back. Each core has five
head SLOTS with fixed k-tile windows (8,8,3,2,2) -- the same program on
every core (SPMD) -- and the host assigns heads to slots so every
head's required window fits. Skipped k-tiles change the at/zz PSUM
accumulation start flags only; ranges are nested so flags stay sound.

Softmax denominator: Z per q via ones-matmul (PSUM), then 1/Z as
exp(-ln Z) on the scalar engine (the DVE reciprocal is ~6 cycles/elem
on one lane), broadcast across partitions with a rank-1 matmul, and a
single DVE multiply into fp16 attnt.

o_proj keeps W_o^T 128x128 blocks stationary and streams attnt tokens
through them (out^T layout, host transposes): each LDWEIGHTS is
amortized over 1024 moving columns and PSUM needs only 2 rotating
banks (blk ping-pong), so the drain runs near peak.

All device-side layouts put the matmul contraction dim on partitions:
  xt    [B, 128, KT, S]        x^T tiles (partition = hidden dim within k-tile)
  wqkv  [3*HPC, 128, KT, 128]  W_pack^T strips per output m-tile
  wo2   [OC2, HPC, 128, 128]   W_o^T blocks (partition = contraction dim)
  out   [OC2, 128, B*S]        out^T partial, fp16 (output dims on partitions)
"""

import math
import os
from contextlib import ExitStack
from dataclasses import dataclass

import numpy as np

import concourse.bass as bass
import concourse.mybir as mybir
from concourse import bacc
import concourse.tile as tile
from concourse import masks
from concourse.bass_utils import run_bass_kernel_spmd

F16 = mybir.dt.float16
F32 = mybir.dt.float32
P = 128
SCALE = 1.0 / math.sqrt(128.0)
LN_PSCALE = math.log(64.0)  # probs scaled by 1/64 so fp16 never overflows
WIN_TOL = 5e-4  # max truncated softmax mass per head
DEFAULT_SLOTS = (8, 8, 3, 2, 2)


@dataclass(frozen=True)
class Cfg:
    B: int = 2
    S: int = 1024
    KT: int = 40  # contraction tiles; H = KT * 128
    HPC: int = 5  # heads per core
    n_cores: int = 8

    @property
    def H(self):
        return self.KT * P

    @property
    def QT(self):
        return self.S // P

    @property
    def MQKV(self):
        return 3 * self.HPC

    @property
    def NBLK(self):
        return self.S // 512

    @property
    def OC2(self):
        return self.H // P


FULL = Cfg()


def build_nc(cfg: Cfg, slots: tuple) -> bass.Bass:
    nc = bacc.Bacc("TRN2", debug=False)
    B, S, KT, HPC, QT, MQKV = cfg.B, cfg.S, cfg.KT, cfg.HPC, cfg.QT, cfg.MQKV
    OC2, NBLK = cfg.OC2, cfg.NBLK
    KPB = 512 // P  # k-tiles per 512-wide q block

    xt_d = nc.dram_tensor("xt", [B, P, KT, S], F16, kind="ExternalInput")
    ws_d = nc.dram_tensor("wqkv", [MQKV, P, KT, P], F16, kind="ExternalInput")
    wo_d = nc.dram_tensor("wo2", [OC2, P, HPC, P], F16, kind="ExternalInput")
    bias_d = nc.dram_tensor("bias", [P, HPC * QT], F32, kind="ExternalInput")
    qramp_d = nc.dram_tensor("qramp", [1, S], F16, kind="ExternalInput")
    slc_d = nc.dram_tensor("slc", [1, HPC * P], F16, kind="ExternalInput")
    out_d = nc.dram_tensor("out", [OC2, P, B * S], F16, kind="ExternalOutput")

    with ExitStack() as ctx:
        tc = ctx.enter_context(tile.TileContext(nc))
        consts = ctx.enter_context(tc.tile_pool(name="consts", bufs=1))
        xt_pool = ctx.enter_context(tc.tile_pool(name="xt", bufs=1))
        wqkv_pool = ctx.enter_context(tc.tile_pool(name="wqkv", bufs=2))
        qkvt_pool = ctx.enter_context(tc.tile_pool(name="qkvt", bufs=2))
        v_pool = ctx.enter_context(tc.tile_pool(name="v", bufs=6))
        p_pool = ctx.enter_context(tc.tile_pool(name="p", bufs=5))
        attnt_pool = ctx.enter_context(tc.tile_pool(name="attnt", bufs=2))
        norm_pool = ctx.enter_context(tc.tile_pool(name="norm", bufs=2))
        vt_pool = ctx.enter_context(tc.tile_pool(name="vt", bufs=2))
        wo_pool = ctx.enter_context(tc.tile_pool(name="wo", bufs=3))
        out_pool = ctx.enter_context(tc.tile_pool(name="out", bufs=4))
        # PSUM budget (8 banks): ps 2 + sc 2 + at 1 + zz 1 + po 2
        ps_pool = ctx.enter_context(tc.tile_pool(name="ps", bufs=2, space="PSUM"))
        sc_pool = ctx.enter_context(tc.tile_pool(name="sc", bufs=2, space="PSUM"))
        at_pool = ctx.enter_context(tc.tile_pool(name="at", bufs=1, space="PSUM"))
        zz_pool = ctx.enter_context(tc.tile_pool(name="zz", bufs=1, space="PSUM"))
        po_pool = ctx.enter_context(tc.tile_pool(name="po", bufs=2, space="PSUM"))

        # constants
        ident = consts.tile([P, P], F16)
        masks.make_identity(nc, ident[:])
        ones = consts.tile([P, 1], F16)
        nc.gpsimd.memset(ones[:], 1.0)
        ones_row = consts.tile([1, P], F16)
        nc.gpsimd.memset(ones_row[:], 1.0)
        bias_sb = consts.tile([P, HPC * QT], F32)
        nc.sync.dma_start(bias_sb[:], bias_d[:])
        qr_sb = consts.tile([1, S], F16)
        nc.sync.dma_start(qr_sb[:], qramp_d[:])
        slc_sb = consts.tile([1, HPC * P], F16)
        nc.sync.dma_start(slc_sb[:], slc_d[:])

        # PE warm-up: self-contained matmuls keep the PE ramping to full
        # p-state while the first input DMAs stream
        warm = ps_pool.tile([P, 512], F32, tag="ps", name="warm")
        for _ in range(40):
            nc.tensor.matmul(warm[:, :P], ident[:], ident[:], start=True, stop=True)

        if KT >= 8:
            sizes = [1, 2, 4, 5]
            rem = KT - sum(sizes)
            nrem = 4
            q, r = divmod(rem, nrem)
            sizes += [q + (1 if i < r else 0) for i in range(nrem)]
        else:
            sizes = [1] * KT
        k2chunk = []
        for ci, s in enumerate(sizes):
            for j in range(s):
                k2chunk.append((ci, j))
        state = {}

        def load_xt(b):
            # chunk tiles with progressive sizes: QKV starts as soon as the
            # first small chunk lands instead of after the full 10MB
            xt_ch = []
            c0 = 0
            for ci, s in enumerate(sizes):
                xc = xt_pool.tile([P, s, S], F16, tag=f"xt{ci}", name=f"xt{ci}")
                nc.sync.dma_start(xc[:], xt_d[b, :, c0 : c0 + s, :])
                xt_ch.append(xc)
                c0 += s
            state[b, "xt"] = xt_ch

        def prefetch_ws(b, m):
            if m >= MQKV:
                b, m = b + 1, 0
            if b >= B:
                return
            ws = wqkv_pool.tile([P, KT, P], F16, tag="ws", name=f"ws{b}_{m}")
            nc.sync.dma_start(ws[:], ws_d[m])
            state[b, "ws", m] = ws

        def qkv_mtile(b, m):
            # one 128-row strip of qkv^T = W^T.T @ x^T (contraction over H);
            # hf-sequential so only one PSUM bank is held at a time
            if (b, "qkvt") not in state:
                state[b, "qkvt"] = qkvt_pool.tile(
                    [P, 2 * HPC, S], F16, tag="qkvt", name=f"qkvt{b}"
                )
            qkvt_sb = state[b, "qkvt"]
            xt_ch = state[b, "xt"]
            ws = state.pop((b, "ws", m))
            prefetch_ws(b, m + 1)
            if m >= 2 * HPC:
                vt = vt_pool.tile([P, S], F16, tag="vt", name=f"vt{b}_{m}")

            def dst_of(hf):
                if m < 2 * HPC:
                    return qkvt_sb[:, m, hf * 512 : (hf + 1) * 512]
                return vt[:, hf * 512 : (hf + 1) * 512]

            if m == 0:
                # first m-tile is paced by the xt DMA: consume each k-tile
                # for both halves as it arrives
                pss = [
                    ps_pool.tile([P, 512], F32, tag="ps", name=f"ps{b}_{m}_{hf}")
                    for hf in range(S // 512)
                ]
                for k in range(KT):
                    ci, cj = k2chunk[k]
                    for hf in range(S // 512):
                        nc.tensor.matmul(
                            pss[hf][:],
                            ws[:, k, :],
                            xt_ch[ci][:, cj, hf * 512 : (hf + 1) * 512],
                            start=(k == 0),
                            stop=(k == KT - 1),
                        )
                    if k % 4 == 3:
                        yield
                for hf in range(S // 512):
                    nc.scalar.activation(
                        dst_of(hf), pss[hf][:], mybir.ActivationFunctionType.Copy
                    )
                yield
            else:
                for hf in range(S // 512):
                    ps = ps_pool.tile(
                        [P, 512], F32, tag="ps", name=f"ps{b}_{m}_{hf}"
                    )
                    for k in range(KT):
                        ci, cj = k2chunk[k]
                        nc.tensor.matmul(
                            ps[:],
                            ws[:, k, :],
                            xt_ch[ci][:, cj, hf * 512 : (hf + 1) * 512],
                            start=(k == 0),
                            stop=(k == KT - 1),
                        )
                        if k % 4 == 3:
                            yield
                    nc.scalar.activation(
                        dst_of(hf), ps[:], mybir.ActivationFunctionType.Copy
                    )
                    yield
            if m >= 2 * HPC:
                # v^T strip: PE-transpose to per-head natural V
                hh = m - 2 * HPC
                v_sb = v_pool.tile([P, QT, P], F16, tag="v", name=f"v{b}_{hh}")
                state[b, "v", hh] = v_sb
                for i in range(QT):
                    tp = sc_pool.tile([P, P], F16, tag="sc")
                    nc.tensor.transpose(tp[:], vt[:, i * P : (i + 1) * P], ident[:])
                    nc.vector.tensor_copy(v_sb[:, i, :], tp[:])
                    if i % 2 == 1:
                        yield

        def finish_norm(b, hh, blk, iz, at_sb):
            # deferred normalize finisher: broadcast 1/Z and multiply; issued
            # a few tiles into the NEXT blk so the PE queue never waits on
            # the reciprocal
            attnt_sb = state[b, "attnt"]
            bc = sc_pool.tile([P, 512], F32, tag="sc")
            nc.tensor.matmul(bc[:], ones_row[:], iz[:], start=True, stop=True)
            izb = norm_pool.tile([P, 512], F16, tag="izb")
            nc.scalar.activation(izb[:], bc[:], mybir.ActivationFunctionType.Copy)
            nc.vector.tensor_tensor(
                attnt_sb[:, hh, blk * 512 : (blk + 1) * 512],
                at_sb[:],
                izb[:],
                mybir.AluOpType.mult,
            )

        def drain_norm():
            pending = state.get("norm_pending")
            if pending:
                finish_norm(*pending)
                state["norm_pending"] = None

        def attn_head(b, hh):
            # scores^T = K^T.T @ Q^T with k-positions on partitions; windowed
            # ragged tiles; p = exp(s/sqrt(d) + alibi_k - slope*q - ln64)
            W = slots[hh]
            if (b, "attnt") not in state:
                state[b, "attnt"] = attnt_pool.tile(
                    [P, HPC, S], F16, tag="attnt", name=f"attnt{b}"
                )
            attnt_sb = state[b, "attnt"]
            qkvt_sb = state[b, "qkvt"]
            v_sb = state[b, "v", hh]
            for blk in range(NBLK):
                i_first = max(0, KPB * blk - W + 1)
                i_last = KPB * (blk + 1) - 1
                at = at_pool.tile([P, 512], F32, tag="at")
                zz = zz_pool.tile([1, 512], F32, tag="zz")
                unflushed = []

                def flush(tile_info):
                    pt, pc0, pw, pi = tile_info
                    o0 = pc0 - blk * 512
                    nc.tensor.matmul(
                        zz[:, o0:512],
                        ones[:],
                        pt[:, :pw],
                        start=(pi == i_first),
                        stop=(pi == i_last),
                    )
                    nc.tensor.matmul(
                        at[:, o0:512],
                        v_sb[:, pi, :],
                        pt[:, :pw],
                        start=(pi == i_first),
                        stop=(pi == i_last),
                    )

                for idx, i in enumerate(range(i_first, i_last + 1)):
                    if idx == 2:
                        drain_norm()
                    k0 = i * P
                    c0 = max(blk * 512, k0)
                    c1 = (blk + 1) * 512
                    w = c1 - c0
                    sc = sc_pool.tile([P, 512], F32, tag="sc")
                    nc.tensor.matmul(
                        sc[:, :w],
                        qkvt_sb[:, HPC + hh, k0 : k0 + P],
                        qkvt_sb[:, hh, c0:c1],
                        start=True,
                        stop=False,
                    )
                    # per-q stabilizer: scores += -slope*q/SCALE (rank-1; any
                    # per-q shift cancels in the softmax normalization)
                    nc.tensor.matmul(
                        sc[:, :w],
                        slc_sb[:, hh * P : (hh + 1) * P],
                        qr_sb[:, c0:c1],
                        start=False,
                        stop=True,
                    )
                    pt = p_pool.tile([P, 512], F16, tag="p")
                    nc.scalar.activation(
                        pt[:, :w],
                        sc[:, :w],
                        mybir.ActivationFunctionType.Exp,
                        bias=bias_sb[:, hh * QT + i : hh * QT + i + 1],
                        scale=SCALE,
                    )
                    if c0 == k0:
                        # diagonal tile: zero probs above the diagonal (on the
                        # idle gpsimd queue; DVE is busy with reciprocals)
                        nc.gpsimd.affine_select(
                            out=pt[:, :P],
                            in_=pt[:, :P],
                            compare_op=mybir.AluOpType.is_ge,
                            fill=0.0,
                            base=0,
                            pattern=[[1, P]],
                            channel_multiplier=-1,
                        )
                    unflushed.append((pt, c0, w, i))
                    if len(unflushed) > 2:
                        flush(unflushed.pop(0))
                    yield
                for t in unflushed:
                    flush(t)
                drain_norm()
                # copy at/zz out of PSUM immediately (frees both banks for
                # the next blk); reciprocal runs on the SBUF copies
                at_sb = norm_pool.tile([P, 512], F16, tag="atsb")
                nc.scalar.activation(
                    at_sb[:], at[:], mybir.ActivationFunctionType.Copy
                )
                zz_sb = norm_pool.tile([1, 512], F32, tag="zzsb")
                nc.scalar.activation(
                    zz_sb[:], zz[:], mybir.ActivationFunctionType.Copy
                )
                iz = norm_pool.tile([1, 512], F16, tag="iz")
                with nc.allow_low_precision("1/Z in f16 is well within tol"):
                    nc.vector.reciprocal(iz[:], zz_sb[:])
                state["norm_pending"] = (b, hh, blk, iz, at_sb)
                yield

        oproj_order = [(0, oc) for oc in range(OC2)] + [
            (1, oc) for oc in range(OC2)
        ]

        def prefetch_wo(idx):
            if idx >= len(oproj_order):
                return
            b, oc = oproj_order[idx]
            wt = wo_pool.tile([P, HPC, P], F16, tag="wo", name=f"wo{b}_{oc}")
            nc.sync.dma_start(wt[:], wo_d[oc])
            state["wo", b, oc] = wt

        def oproj_chunk(idx):
            # out^T[oc, tok] partial: W_o^T blocks stationary, attnt moving
            b, oc = oproj_order[idx]
            attnt_sb = state[b, "attnt"]
            wt = state.pop(("wo", b, oc))
            prefetch_wo(idx + 2)
            for blk in range(NBLK):
                po = po_pool.tile([P, 512], F32, tag="po")
                for k in range(HPC):
                    nc.tensor.matmul(
                        po[:],
                        wt[:, k, :],
                        attnt_sb[:, k, blk * 512 : (blk + 1) * 512],
                        start=(k == 0),
                        stop=(k == HPC - 1),
                    )
                    if k == 2:
                        yield
                ot = out_pool.tile([P, 512], F16, tag="ot")
                if blk % 2 == 0:
                    nc.scalar.activation(
                        ot[:], po[:], mybir.ActivationFunctionType.Copy
                    )
                else:
                    nc.vector.tensor_copy(ot[:], po[:])
                nc.sync.dma_start(
                    out_d[oc, :, b * S + blk * 512 : b * S + (blk + 1) * 512],
                    ot[:],
                )
                yield

        class Stepper:
            def __init__(self, gens):
                self.gens = list(gens)
                self.i = 0

            def step(self):
                while self.i < len(self.gens):
                    try:
                        next(self.gens[self.i])
                        return True
                    except StopIteration:
                        self.i += 1
                return False

            def drain(self):
                while self.step():
                    pass

        def weave(primaries, filler, ratio):
            for g in primaries:
                for _ in g:
                    for _ in range(ratio):
                        if not filler.step():
                            break

        # ---- software pipeline
        prefetch_ws(0, 0)
        load_xt(0)
        Stepper([qkv_mtile(0, m) for m in range(MQKV)]).drain()
        load_xt(1)
        qkv1 = Stepper([qkv_mtile(1, m) for m in range(MQKV)])
        weave([attn_head(0, hh) for hh in range(HPC)], qkv1, ratio=2)
        drain_norm()
        prefetch_wo(0)
        prefetch_wo(1)
        weave([oproj_chunk(i) for i in range(OC2 - 28)], qkv1, ratio=1)
        qkv1.drain()
        tail0 = Stepper([oproj_chunk(i) for i in range(OC2 - 28, OC2)])
        weave([attn_head(1, hh) for hh in range(HPC)], tail0, ratio=2)
        drain_norm()
        tail0.drain()
        Stepper([oproj_chunk(i) for i in range(OC2, 2 * OC2)]).drain()

    nc.compile()
    return nc


def head_windows(slopes):
    """Per-head minimum k-tile window (2/3/4) or 8 (full) from alibi slopes."""
    req = []
    for s in slopes:
        w = 8
        if s > 0:
            for cand in (2, 3, 4):
                g = 128 * cand - 127
                mass = math.exp(-g * s) / max(1e-30, 1.0 - math.exp(-s))
                if mass <= WIN_TOL:
                    w = cand
                    break
        req.append(w)
    return req


def assign_heads(slopes, cfg: Cfg, slot_shape=DEFAULT_SLOTS):
    """Assign heads to per-core slots so each head's window fits its slot.
    Returns (heads_per_core, slots) -- falls back to full windows if the
    slopes don't fit the default slot shape."""
    NH = len(slopes)
    req = head_windows(slopes)
    for slots in (slot_shape, (8,) * cfg.HPC):
        all_slots = [
            (c, k) for c in range(cfg.n_cores) for k in range(cfg.HPC)
        ]
        # place most demanding heads first, into the tightest fitting slot
        order = sorted(range(NH), key=lambda h: -req[h])
        free = sorted(all_slots, key=lambda s: slots[s[1]])
        placement = {}
        ok = True
        for h in order:
            pick = None
            for idx, (c, k) in enumerate(free):
                if slots[k] >= req[h]:
                    pick = idx
                    break
            if pick is None:
                ok = False
                break
            placement[free.pop(pick)] = h
        if ok:
            heads_per_core = [
                [placement[(c, k)] for k in range(cfg.HPC)]
                for c in range(cfg.n_cores)
            ]
            return heads_per_core, tuple(slots)
    raise AssertionError("unreachable: full windows always fit")


def prep_inputs(hidden_states, W_pack, W_o, attention_mask, cfg: Cfg = FULL):
    """Shard + lay out the full inputs for the 8 cores."""
    B, S, KT, HPC = cfg.B, cfg.S, cfg.KT, cfg.HPC
    H = cfg.H
    hs = np.asarray(hidden_states)
    wp = np.asarray(W_pack)
    wo = np.asarray(W_o)
    am = np.asarray(attention_mask)

    # x^T layout [B, 128, KT, S]: xt[b, p, k, t] = hs[b, t, k*128 + p]
    xt = np.ascontiguousarray(
        hs.reshape(B, S, KT, P).transpose(0, 3, 2, 1).astype(np.float16)
    )

    # alibi slopes from the mask: mask[h, q, k] = causal + slope_h * k
    slopes = am[:, -1, 1].astype(np.float64)  # mask[h, S-1, 1] = slope_h
    if os.environ.get("BAI_NOWIN"):
        slots = (8,) * cfg.HPC
        heads_per_core = [
            list(range(c * cfg.HPC, (c + 1) * cfg.HPC))
            for c in range(cfg.n_cores)
        ]
    else:
        heads_per_core, slots = assign_heads(slopes, cfg)

    kvec = np.arange(P, dtype=np.float64)
    in_maps = []
    for c in range(cfg.n_cores):
        heads = heads_per_core[c]
        # W_pack^T strips: m-tiles [q0..q4, k0..k4, v0..v4] for this core's heads
        rows = []
        for sec in range(3):  # q, k, v blocks of W_pack
            for h in heads:
                r0 = sec * H + h * P
                rows.append(wp[r0 : r0 + P, :])  # [128, H]
        # strip[m, p, k, j] = W_pack[row_j, k*128 + p]
        ws = np.stack(
            [r.T.reshape(KT, P, P).transpose(1, 0, 2) for r in rows]
        ).astype(np.float16)

        # W_o^T blocks: wo2[oc, p, k, o] = W_o[oc*128 + o, heads[k]*128 + p]
        # (dram layout matches the SBUF tile [P, HPC, P] exactly)
        wo2 = np.empty((cfg.OC2, P, HPC, P), dtype=np.float16)
        for k, h in enumerate(heads):
            cols = wo[:, h * P : (h + 1) * P]  # [H, 128]
            wo2[:, :, k, :] = cols.reshape(cfg.OC2, P, P).transpose(0, 2, 1)

        # exp bias table [128, HPC*QT]: col hh*QT + i -> slope*(i*128+k) - lnPS
        bias = np.empty((P, HPC * cfg.QT), dtype=np.float32)
        slc = np.empty((1, HPC * P), dtype=np.float16)
        for hh, h in enumerate(heads):
            for i in range(cfg.QT):
                bias[:, hh * cfg.QT + i] = (
                    slopes[h] * (i * P + kvec) - LN_PSCALE
                ).astype(np.float32)
            slc[0, hh * P : (hh + 1) * P] = np.float16(slopes[h])
        qramp = (
            -np.arange(S, dtype=np.float64) * math.sqrt(128.0)
        ).astype(np.float16)[None, :]

        in_maps.append(
            {
                "xt": xt,
                "wqkv": np.ascontiguousarray(ws),
                "wo2": np.ascontiguousarray(wo2),
                "bias": bias,
                "qramp": qramp,
                "slc": slc,
            }
        )
    return in_maps, slots


_CACHE = {}


def _get_nc(cfg: Cfg, slots: tuple) -> bass.Bass:
    key = (cfg, slots)
    if key not in _CACHE:
        _CACHE[key] = build_nc(cfg, slots)
    return _CACHE[key]


def run(hidden_states, W_pack, W_o, attention_mask, cfg: Cfg = FULL, **kw):
    in_maps, slots = prep_inputs(hidden_states, W_pack, W_o, attention_mask, cfg)
    nc = _get_nc(cfg, slots)
    res = run_bass_kernel_spmd(nc, in_maps, core_ids=list(range(cfg.n_cores)), **kw)
    # sum the per-core partials (fp16 -> fp32), unshard out^T to [B, S, H]
    acc = np.zeros((cfg.OC2, P, cfg.B * cfg.S), dtype=np.float32)
    for r in res.results:
        acc += r["out"].astype(np.float32)
    out = acc.reshape(cfg.H, cfg.B, cfg.S).transpose(1, 2, 0)
    return np.ascontiguousarray(out), res


def kernel(hidden_states, W_pack, W_o, attention_mask):
    out, _ = run(hidden_states, W_pack, W_o, attention_mask)
    return out.astype(np.float32)
